# revision 12
# baseline (speedup 1.0000x reference)
"""Affinity-propagate (SPN) Trainium2 Bass kernel.

Computation (per batch element, see reference):
    w = g / conv3x3_ones(|g|)          # gates, [8, H, W], computed once
    d_{k+1} = max_c conv3x3_ones(w_c * d_k)   # 8 iterations

Distribution: pure data parallel, batch element b -> NeuronCore b (8 cores).

Per-core mapping:
  - H=352 rows live on SBUF partitions as 3 overlapping tiles
    (rows 0..127, 126..253, 252..351).  The 3x3 conv's H-direction sum is a
    tri-band matrix matmul on the tensor engine (contraction over the
    partition/H axis); output rows at tile seams that lack a cross-tile
    neighbour are invalid and are instead produced by the adjacent tile, with
    4 one-row SBUF->SBUF DMA "seam" copies per iteration.
  - The W-direction sum is folded into the same matmuls: 3 PSUM-accumulating
    matmuls with the moving operand shifted by -1/0/+1 columns (W is padded
    by one zero column on each side).
  - Work split: DVE computes p = w*d (fp32 in, float32r out, one op per
    (tile, channel) plane) and the channel-max tree; PE does all conv sums
    (float32r, 1 cycle/col at N>=256); ScalarE evacuates PSUM->SBUF;
    DMA does seam rows.
  - d is stored fp32, updated in place (trace order makes WAR/RAW safe);
    only p is rounded to float32r (~1e-4 relative per conv), keeping the
    final relative error ~3e-4.
"""
from contextlib import ExitStack

import numpy as np

import concourse.bacc as bacc
import concourse.mybir as mybir
import concourse.tile as tile
from concourse.bass_utils import run_bass_kernel_spmd

F32 = mybir.dt.float32
F32R = mybir.dt.float32r

B, C, H, W = 8, 8, 352, 1216
WB = W + 2  # zero-padded width
N_ITERS = 8
N_CORES = 8
SKIP_SEAMS = False
P_BUFS = 5
PROP_BUFS = 5

ROW_BASE = [0, 126, 252]       # first global row of each H tile
ROWS = [128, 128, 100]         # partitions used by each H tile
CHUNKS = [(0, 512), (512, 448), (960, 256)]  # (start col, width); >=256 for f32r speed


def _build_nc():
    nc = bacc.Bacc("TRN2", target_bir_lowering=False, debug=False,
                   num_devices=N_CORES)
    g = nc.dram_tensor("g", [C, H, W], F32, kind="ExternalInput").ap()
    d_in = nc.dram_tensor("d", [H, W], F32, kind="ExternalInput").ap()
    band = nc.dram_tensor("band", [128, 128], F32R, kind="ExternalInput").ap()
    out = nc.dram_tensor("out", [H, W], F32, kind="ExternalOutput").ap()

    with tile.TileContext(nc) as tc, ExitStack() as ctx:
        pw = ctx.enter_context(tc.tile_pool(name="w", bufs=1))
        pd = ctx.enter_context(tc.tile_pool(name="d", bufs=1))
        pc = ctx.enter_context(tc.tile_pool(name="const", bufs=1))
        pp = ctx.enter_context(tc.tile_pool(name="p", bufs=P_BUFS))
        pprop = ctx.enter_context(tc.tile_pool(name="prop", bufs=PROP_BUFS))
        ptree1 = ctx.enter_context(tc.tile_pool(name="tree1", bufs=2))
        prm = ctx.enter_context(tc.tile_pool(name="rm", bufs=2))
        psum = ctx.enter_context(tc.tile_pool(name="psum", bufs=8, space="PSUM"))

        A = pc.tile([128, 128], F32R, tag="band", name="bandt")
        nc.sync.dma_start(A[:], band[:])

        wt = [pw.tile([128, C, WB], F32, tag=f"w{t}", name=f"w{t}")
              for t in range(3)]
        dt_ = [pd.tile([128, WB], F32, tag=f"d{t}", name=f"d{t}")
               for t in range(3)]

        # ---- zero pad columns, load inputs ----
        for t in range(3):
            R, rb = ROWS[t], ROW_BASE[t]
            nc.vector.memset(wt[t][:, :, 0:1], 0.0)
            nc.vector.memset(wt[t][:, :, WB - 1:WB], 0.0)
            nc.vector.memset(dt_[t][:, 0:1], 0.0)
            nc.vector.memset(dt_[t][:, WB - 1:WB], 0.0)
            nc.sync.dma_start(
                wt[t][0:R, :, 1:W + 1],
                g[:, rb:rb + R, :].rearrange("c r w -> r c w"))
            nc.sync.dma_start(dt_[t][0:R, 1:W + 1], d_in[rb:rb + R, :])

        # ---- phase 0: w = g / conv3x3_ones(|g|) ----
        for t in range(3):
            R = ROWS[t]
            for c in range(C):
                p = pp.tile([128, WB], F32R, tag="p", name="p")
                nc.scalar.activation(p[0:R, :], wt[t][0:R, c, :],
                                     mybir.ActivationFunctionType.Abs)
                s_buf = ptree1.tile([128, W], F32, tag="t1", name="sbuf_")
                for (J, N) in CHUNKS:
                    ps = psum.tile([128, 512], F32, tag="ps", name="ps")
                    for s in range(3):
                        nc.tensor.matmul(ps[0:R, 0:N], A[0:R, 0:R],
                                         p[0:R, J + s:J + s + N],
                                         start=(s == 0), stop=(s == 2))
                    nc.scalar.copy(s_buf[0:R, J:J + N], ps[0:R, 0:N])
                rcp = ptree1.tile([128, W], F32, tag="t1", name="rcp")
                nc.vector.reciprocal_approx_fast(out=rcp[0:R, :],
                                                 in_=s_buf[0:R, :])
                nc.vector.tensor_mul(wt[t][0:R, c, 1:W + 1],
                                     wt[t][0:R, c, 1:W + 1], rcp[0:R, :])
        # w seam rows
        nc.sync.dma_start(wt[0][127:128, :, 1:W + 1], wt[1][1:2, :, 1:W + 1])
        nc.sync.dma_start(wt[1][0:1, :, 1:W + 1], wt[0][126:127, :, 1:W + 1])
        nc.sync.dma_start(wt[1][127:128, :, 1:W + 1], wt[2][1:2, :, 1:W + 1])
        nc.sync.dma_start(wt[2][0:1, :, 1:W + 1], wt[1][126:127, :, 1:W + 1])

        # ---- phase 1: 8 propagation iterations ----
        for k in range(N_ITERS):
            for t in range(3):
                R = ROWS[t]
                props = []
                for c in range(C):
                    p = pp.tile([128, WB], F32R, tag="p", name="p")
                    nc.vector.tensor_mul(p[0:R, :], wt[t][0:R, c, :],
                                         dt_[t][0:R, :])
                    prop = pprop.tile([128, W], F32, tag="prop", name="prop")
                    for (J, N) in CHUNKS:
                        ps = psum.tile([128, 512], F32, tag="ps", name="ps")
                        for s in range(3):
                            nc.tensor.matmul(ps[0:R, 0:N], A[0:R, 0:R],
                                             p[0:R, J + s:J + s + N],
                                             start=(s == 0), stop=(s == 2))
                        nc.scalar.copy(prop[0:R, J:J + N], ps[0:R, 0:N])
                    props.append(prop)
                    # incremental channel max; last step writes d in place
                    # (junk seam rows fixed by the seam DMAs below)
                    if c == 1:
                        rm = prm.tile([128, W], F32, tag="rm", name="rm")
                        nc.vector.tensor_max(rm[0:R, :], props[0][0:R, :],
                                             props[1][0:R, :])
                    elif c in (2, 3, 4, 5, 6):
                        nc.vector.tensor_max(rm[0:R, :], rm[0:R, :],
                                             props[c][0:R, :])
                    elif c == 7:
                        nc.vector.tensor_max(dt_[t][0:R, 1:W + 1],
                                             rm[0:R, :], props[7][0:R, :])
            # seam rows
            if not SKIP_SEAMS:
                nc.sync.dma_start(dt_[0][127:128, 1:W + 1], dt_[1][1:2, 1:W + 1])
                nc.sync.dma_start(dt_[1][0:1, 1:W + 1], dt_[0][126:127, 1:W + 1])
                nc.sync.dma_start(dt_[1][127:128, 1:W + 1], dt_[2][1:2, 1:W + 1])
                nc.sync.dma_start(dt_[2][0:1, 1:W + 1], dt_[1][126:127, 1:W + 1])

        nc.sync.dma_start(out[0:128, :], dt_[0][0:128, 1:W + 1])
        nc.sync.dma_start(out[128:254, :], dt_[1][2:128, 1:W + 1])
        nc.sync.dma_start(out[254:352, :], dt_[2][2:100, 1:W + 1])

    nc.compile()
    return nc


def _band_matrix():
    a = np.zeros((128, 128), dtype=np.float32)
    idx = np.arange(128)
    a[idx, idx] = 1.0
    a[idx[:-1], idx[:-1] + 1] = 1.0
    a[idx[1:], idx[1:] - 1] = 1.0
    return a


_NC_CACHE = None


def kernel(guidance: np.ndarray, blur_depth: np.ndarray) -> np.ndarray:
    """Full inputs in, full output out. Shards batch across 8 NeuronCores."""
    global _NC_CACHE
    guidance = np.asarray(guidance, dtype=np.float32)
    blur_depth = np.asarray(blur_depth, dtype=np.float32)
    assert guidance.shape == (B, C, H, W), guidance.shape
    assert blur_depth.shape == (B, 1, H, W), blur_depth.shape
    if _NC_CACHE is None:
        _NC_CACHE = _build_nc()
    nc = _NC_CACHE
    band = _band_matrix()
    in_maps = [
        {
            "g": np.ascontiguousarray(guidance[b], dtype=np.float32),
            "d": np.ascontiguousarray(blur_depth[b, 0], dtype=np.float32),
            "band": band,
        }
        for b in range(B)
    ]
    res = run_bass_kernel_spmd(nc, in_maps, core_ids=list(range(N_CORES)))
    out = np.stack([res.results[b]["out"] for b in range(B)])[:, None]
    return out.astype(np.float32)


# revision 16
# speedup vs baseline: 3.2442x; 3.2442x over previous
"""Affinity-propagate (SPN) Trainium2 Bass kernel.

Computation (per batch element, see reference):
    w = g / conv3x3_ones(|g|)          # gates, [8, H, W], computed once
    d_{k+1} = max_c conv3x3_ones(w_c * d_k)   # 8 iterations

Distribution: pure data parallel, batch element b -> NeuronCore b (8 cores).

Per-core mapping:
  - H=352 rows live on SBUF partitions as 3 overlapping tiles
    (rows 0..127, 126..253, 252..351).  The 3x3 conv's H-direction sum is a
    tri-band matrix matmul on the tensor engine (contraction over the
    partition/H axis); output rows at tile seams that lack a cross-tile
    neighbour are invalid and are instead produced by the adjacent tile, with
    4 one-row SBUF->SBUF DMA "seam" copies per iteration.
  - The W-direction sum is folded into the same matmuls: 3 PSUM-accumulating
    matmuls with the moving operand shifted by -1/0/+1 columns (W is padded
    by one zero column on each side).
  - Work split: DVE computes p = w*d (fp32 in, float32r out, one op per
    (tile, channel) plane) and the channel-max tree; PE does all conv sums
    (float32r, 1 cycle/col at N>=256); ScalarE evacuates PSUM->SBUF;
    DMA does seam rows.
  - d is stored fp32, updated in place (trace order makes WAR/RAW safe);
    only p is rounded to float32r (~1e-4 relative per conv), keeping the
    final relative error ~3e-4.

Measured (8x trn2 NeuronCores via axon):
  - relative error vs fp32 jax reference: 3.03e-4
  - device execution: ~38-48 us per propagation iteration (N_ITERS=32 vs 64
    scaling), i.e. ~350-400 us for the full 8-iteration kernel; the
    dispatch-inclusive per-call marginal through the axon tunnel is ~0.9 ms.
"""
from contextlib import ExitStack

import numpy as np

import concourse.bacc as bacc
import concourse.mybir as mybir
import concourse.tile as tile
from concourse.bass_utils import run_bass_kernel_spmd

F32 = mybir.dt.float32
F32R = mybir.dt.float32r

B, C, H, W = 8, 8, 352, 1216
WB = W + 2  # zero-padded width
N_ITERS = 8
N_CORES = 8
SKIP_SEAMS = False
P_BUFS = 5
PROP_BUFS = 6
T1_BUFS = 2
RM_BUFS = 2

ROW_BASE = [0, 126, 252]       # first global row of each H tile
ROWS = [128, 128, 100]         # partitions used by each H tile
CHUNKS = [(0, 512), (512, 448), (960, 256)]  # (start col, width); >=256 for f32r speed


def _build_nc():
    nc = bacc.Bacc("TRN2", target_bir_lowering=False, debug=False,
                   num_devices=N_CORES)
    g = nc.dram_tensor("g", [C, H, W], F32, kind="ExternalInput").ap()
    d_in = nc.dram_tensor("d", [H, W], F32, kind="ExternalInput").ap()
    band = nc.dram_tensor("band", [128, 128], F32R, kind="ExternalInput").ap()
    out = nc.dram_tensor("out", [H, W], F32, kind="ExternalOutput").ap()

    with tile.TileContext(nc) as tc, ExitStack() as ctx:
        pw = ctx.enter_context(tc.tile_pool(name="w", bufs=1))
        pd = ctx.enter_context(tc.tile_pool(name="d", bufs=1))
        pc = ctx.enter_context(tc.tile_pool(name="const", bufs=1))
        pp = ctx.enter_context(tc.tile_pool(name="p", bufs=P_BUFS))
        pprop = ctx.enter_context(tc.tile_pool(name="prop", bufs=PROP_BUFS))
        ptree1 = ctx.enter_context(tc.tile_pool(name="tree1", bufs=T1_BUFS))
        prm = ctx.enter_context(tc.tile_pool(name="rm", bufs=RM_BUFS))
        psum = ctx.enter_context(tc.tile_pool(name="psum", bufs=8, space="PSUM"))

        A = pc.tile([128, 128], F32R, tag="band", name="bandt")
        nc.sync.dma_start(A[:], band[:])

        wt = [pw.tile([128, C, WB], F32, tag=f"w{t}", name=f"w{t}")
              for t in range(3)]
        dt_ = [pd.tile([128, WB], F32, tag=f"d{t}", name=f"d{t}")
               for t in range(3)]

        # ---- zero pad columns, load inputs ----
        for t in range(3):
            R, rb = ROWS[t], ROW_BASE[t]
            nc.vector.memset(wt[t][:, :, 0:1], 0.0)
            nc.vector.memset(wt[t][:, :, WB - 1:WB], 0.0)
            nc.vector.memset(dt_[t][:, 0:1], 0.0)
            nc.vector.memset(dt_[t][:, WB - 1:WB], 0.0)
            nc.sync.dma_start(
                wt[t][0:R, :, 1:W + 1],
                g[:, rb:rb + R, :].rearrange("c r w -> r c w"))
            nc.sync.dma_start(dt_[t][0:R, 1:W + 1], d_in[rb:rb + R, :])

        # ---- phase 0: w = g / conv3x3_ones(|g|) ----
        for t in range(3):
            R = ROWS[t]
            for c in range(C):
                p = pp.tile([128, WB], F32R, tag="p", name="p")
                nc.scalar.activation(p[0:R, :], wt[t][0:R, c, :],
                                     mybir.ActivationFunctionType.Abs)
                s_buf = ptree1.tile([128, W], F32, tag="t1", name="sbuf_")
                for (J, N) in CHUNKS:
                    ps = psum.tile([128, 512], F32, tag="ps", name="ps")
                    for s in range(3):
                        nc.tensor.matmul(ps[0:R, 0:N], A[0:R, 0:R],
                                         p[0:R, J + s:J + s + N],
                                         start=(s == 0), stop=(s == 2))
                    nc.scalar.copy(s_buf[0:R, J:J + N], ps[0:R, 0:N])
                rcp = ptree1.tile([128, W], F32, tag="t1", name="rcp")
                nc.vector.reciprocal_approx_fast(out=rcp[0:R, :],
                                                 in_=s_buf[0:R, :])
                nc.vector.tensor_mul(wt[t][0:R, c, 1:W + 1],
                                     wt[t][0:R, c, 1:W + 1], rcp[0:R, :])
        # w seam rows
        nc.sync.dma_start(wt[0][127:128, :, 1:W + 1], wt[1][1:2, :, 1:W + 1])
        nc.sync.dma_start(wt[1][0:1, :, 1:W + 1], wt[0][126:127, :, 1:W + 1])
        nc.sync.dma_start(wt[1][127:128, :, 1:W + 1], wt[2][1:2, :, 1:W + 1])
        nc.sync.dma_start(wt[2][0:1, :, 1:W + 1], wt[1][126:127, :, 1:W + 1])

        # ---- phase 1: 8 propagation iterations ----
        for k in range(N_ITERS):
            for t in range(3):
                R = ROWS[t]
                props = []
                for c in range(C):
                    p = pp.tile([128, WB], F32R, tag="p", name="p")
                    nc.vector.tensor_mul(p[0:R, :], wt[t][0:R, c, :],
                                         dt_[t][0:R, :])
                    prop = pprop.tile([128, W], F32, tag="prop", name="prop")
                    for (J, N) in CHUNKS:
                        ps = psum.tile([128, 512], F32, tag="ps", name="ps")
                        for s in range(3):
                            nc.tensor.matmul(ps[0:R, 0:N], A[0:R, 0:R],
                                             p[0:R, J + s:J + s + N],
                                             start=(s == 0), stop=(s == 2))
                        nc.scalar.copy(prop[0:R, J:J + N], ps[0:R, 0:N])
                    props.append(prop)
                    # incremental channel max; last step writes d in place
                    # (junk seam rows fixed by the seam DMAs below)
                    if c == 1:
                        rm = prm.tile([128, W], F32, tag="rm", name="rm")
                        nc.vector.tensor_max(rm[0:R, :], props[0][0:R, :],
                                             props[1][0:R, :])
                    elif c in (2, 3, 4, 5, 6):
                        nc.vector.tensor_max(rm[0:R, :], rm[0:R, :],
                                             props[c][0:R, :])
                    elif c == 7:
                        nc.vector.tensor_max(dt_[t][0:R, 1:W + 1],
                                             rm[0:R, :], props[7][0:R, :])
            # seam rows
            if not SKIP_SEAMS:
                nc.sync.dma_start(dt_[0][127:128, 1:W + 1], dt_[1][1:2, 1:W + 1])
                nc.sync.dma_start(dt_[1][0:1, 1:W + 1], dt_[0][126:127, 1:W + 1])
                nc.sync.dma_start(dt_[1][127:128, 1:W + 1], dt_[2][1:2, 1:W + 1])
                nc.sync.dma_start(dt_[2][0:1, 1:W + 1], dt_[1][126:127, 1:W + 1])

        nc.sync.dma_start(out[0:128, :], dt_[0][0:128, 1:W + 1])
        nc.sync.dma_start(out[128:254, :], dt_[1][2:128, 1:W + 1])
        nc.sync.dma_start(out[254:352, :], dt_[2][2:100, 1:W + 1])

    nc.compile()
    return nc


def _band_matrix():
    a = np.zeros((128, 128), dtype=np.float32)
    idx = np.arange(128)
    a[idx, idx] = 1.0
    a[idx[:-1], idx[:-1] + 1] = 1.0
    a[idx[1:], idx[1:] - 1] = 1.0
    return a


_NC_CACHE = None


def kernel(guidance: np.ndarray, blur_depth: np.ndarray) -> np.ndarray:
    """Full inputs in, full output out. Shards batch across 8 NeuronCores."""
    global _NC_CACHE
    guidance = np.asarray(guidance, dtype=np.float32)
    blur_depth = np.asarray(blur_depth, dtype=np.float32)
    assert guidance.shape == (B, C, H, W), guidance.shape
    assert blur_depth.shape == (B, 1, H, W), blur_depth.shape
    if _NC_CACHE is None:
        _NC_CACHE = _build_nc()
    nc = _NC_CACHE
    band = _band_matrix()
    in_maps = [
        {
            "g": np.ascontiguousarray(guidance[b], dtype=np.float32),
            "d": np.ascontiguousarray(blur_depth[b, 0], dtype=np.float32),
            "band": band,
        }
        for b in range(B)
    ]
    res = run_bass_kernel_spmd(nc, in_maps, core_ids=list(range(N_CORES)))
    out = np.stack([res.results[b]["out"] for b in range(B)])[:, None]
    return out.astype(np.float32)
